# revision 11
# baseline (speedup 1.0000x reference)
"""Trainium2 Bass kernel for the MoE layer (router top-2 + 8 experts + residual LN).

Strategy (token-parallel, all math on device, no collectives):
  - The 16384 tokens are split into 8 blocks of 2048, one per NeuronCore.
  - Host does integer-only routing bookkeeping: it computes router logits in
    numpy just to pick each token's top-2 expert indices, then builds a
    grouped/padded gather of each core's tokens by expert (capacity CG per
    (core, expert) group) plus gather indices for the combine step.
  - Each core, on device:
      phase 1 (router): logits = x_blk @ Wr.T in fp32 on the PE, top-2 values
        via DVE max8, combine weights wA = sigmoid(v1 - v2), wB = 1 - wA.
      phase 2 (experts): for each expert e, h = gelu(W1[e].T @ xgT + b1[e])
        (bf16 matmuls, fp32 PSUM accumulate, gelu+bias fused on ScalarE),
        y = h.T @ W2[e] emitted token-major straight from the PE by using the
        h tile as the stationary operand; y rows stored bf16 to DRAM.
      phase 3 (combine + LN): per 128-token tile, indirect-DMA gather of the
        two contribution rows per token, res = x + b2 + wA*gA + wB*gB, then
        LayerNorm (bn_stats/bn_aggr) * gamma + beta -> out block fp32.
  - Host concatenates the 8 output blocks.
"""

import sys

sys.path.insert(0, "/opt/trn_rl_repo")

import numpy as np
import ml_dtypes

import concourse.bass as bass
import concourse.mybir as mybir
import concourse.tile as tile
from concourse import bacc
from concourse import bass_utils
from concourse.bass import ts

P = 128
B, S, H, E = 8, 2048, 1024, 8
T = B * S
NCORES = 8
TBLK = T // NCORES  # 2048 tokens per core
D2 = 2 * H  # 2048
LN_EPS = 1e-5
KH = H // P  # 8 k-chunks over H
K2 = D2 // P  # 16 k-chunks over 2H
M2 = D2 // P  # 16 feature chunks of the hidden layer

BF16 = mybir.dt.bfloat16
F32 = mybir.dt.float32
I32 = mybir.dt.int32
AFT = mybir.ActivationFunctionType
ALU = mybir.AluOpType


def _chunks(total, step):
    out = []
    off = 0
    while off < total:
        sz = min(step, total - off)
        out.append((off, sz))
        off += sz
    return out


def _bcast_row(ap, parts):
    """A [D] DRAM AP broadcast to [parts, D] (partition step 0)."""
    return bass.AP(tensor=ap.tensor, offset=ap.offset, ap=[[0, parts], *ap.ap])


def build_moe_nc(tblk=TBLK, cg=576, enable_asserts=False, debug_taps=False):
    """Build + compile the per-core Bass program. Same program runs SPMD on
    all 8 cores; per-core behavior differs only through input data."""
    nt = tblk // P  # token tiles per core
    c_rows = E * cg  # FFN rows per core

    nc = bacc.Bacc(
        "TRN2",
        target_bir_lowering=False,
        debug=False,
        enable_asserts=enable_asserts,
        num_devices=NCORES,
    )

    xgT = nc.dram_tensor("xgT", [H, c_rows], BF16, kind="ExternalInput").ap()
    xblkT = nc.dram_tensor("xblkT", [H, tblk], F32, kind="ExternalInput").ap()
    xblk = nc.dram_tensor("xblk", [tblk, H], F32, kind="ExternalInput").ap()
    WrT = nc.dram_tensor("WrT", [H, E], F32, kind="ExternalInput").ap()
    W1 = nc.dram_tensor("W1", [E, H, D2], BF16, kind="ExternalInput").ap()
    W2 = nc.dram_tensor("W2", [E, D2, H], BF16, kind="ExternalInput").ap()
    b1 = nc.dram_tensor("b1", [E, D2], F32, kind="ExternalInput").ap()
    b2 = nc.dram_tensor("b2", [E, H], F32, kind="ExternalInput").ap()
    gamma = nc.dram_tensor("gamma", [H], F32, kind="ExternalInput").ap()
    beta = nc.dram_tensor("beta", [H], F32, kind="ExternalInput").ap()
    idxA = nc.dram_tensor("idxA", [P, nt], I32, kind="ExternalInput").ap()
    idxB = nc.dram_tensor("idxB", [P, nt], I32, kind="ExternalInput").ap()
    out = nc.dram_tensor("out", [tblk, H], F32, kind="ExternalOutput").ap()
    if debug_taps:
        y_dbg = nc.dram_tensor("y_dbg", [E * cg, H], BF16, kind="ExternalOutput").ap()
        wA_dbg = nc.dram_tensor("wA_dbg", [P, nt], F32, kind="ExternalOutput").ap()
        wB_dbg = nc.dram_tensor("wB_dbg", [P, nt], F32, kind="ExternalOutput").ap()
        lg_dbg = nc.dram_tensor("lg_dbg", [P, nt, E], F32, kind="ExternalOutput").ap()

    with tile.TileContext(nc) as tc:
        with (
            tc.tile_pool(name="persist", bufs=1) as persist,
            tc.tile_pool(name="dram", bufs=1, space="DRAM") as dram,
        ):
            y_dram = dram.tile([c_rows, H], BF16)

            wA_sb = persist.tile([P, nt], F32)
            wB_sb = persist.tile([P, nt], F32)
            gam_bc = persist.tile([P, H], F32)
            bet_bc = persist.tile([P, H], F32)
            eps_t = persist.tile([P, 1], F32)
            nc.vector.memset(eps_t[:], LN_EPS)
            nc.sync.dma_start(gam_bc[:], _bcast_row(gamma, P))
            nc.sync.dma_start(bet_bc[:], _bcast_row(beta, P))

            # ---------------- phase 1: router ----------------
            with (
                tc.tile_pool(name="rt", bufs=3) as rpool,
                tc.tile_pool(name="rps", bufs=2, space="PSUM") as rpsum,
            ):
                wrt = persist.tile([P, KH, E], F32)
                nc.sync.dma_start(wrt[:], WrT.rearrange("(ko p) e -> p ko e", p=P))
                for i in range(nt):
                    xbt = rpool.tile([P, KH, P], F32)
                    nc.sync.dma_start(
                        xbt[:],
                        xblkT[:, ts(i, P)].rearrange("(ko p) n -> p ko n", p=P),
                    )
                    ps = rpsum.tile([P, E], F32)
                    for k in range(KH):
                        nc.tensor.matmul(
                            ps[:],
                            lhsT=xbt[:, k, :],
                            rhs=wrt[:, k, :],
                            start=(k == 0),
                            stop=(k == KH - 1),
                        )
                    mx = rpool.tile([P, 8], F32)
                    nc.vector.max(mx[:], ps[:])
                    if debug_taps:
                        lgs = rpool.tile([P, E], F32)
                        nc.vector.tensor_copy(lgs[:], ps[:])
                        nc.sync.dma_start(lg_dbg[:, i, :], lgs[:])
                    d = rpool.tile([P, 1], F32)
                    nc.vector.tensor_sub(d[:], mx[:, 0:1], mx[:, 1:2])
                    nc.scalar.activation(wA_sb[:, i : i + 1], d[:], AFT.Sigmoid)
                    # wB = 1 - wA  (matches softmax over the top-2 logits)
                    nc.vector.tensor_scalar(
                        wB_sb[:, i : i + 1],
                        wA_sb[:, i : i + 1],
                        -1.0,
                        1.0,
                        op0=ALU.mult,
                        op1=ALU.add,
                    )

            # ---------------- phase 2: experts ----------------
            with (
                tc.tile_pool(name="w1p", bufs=2) as w1p,
                tc.tile_pool(name="w2p", bufs=2) as w2p,
                tc.tile_pool(name="xgp", bufs=2) as xgp,
                tc.tile_pool(name="hp", bufs=2) as hp,
                tc.tile_pool(name="ysb", bufs=4) as ysb,
                tc.tile_pool(name="b1p", bufs=2) as b1p,
                tc.tile_pool(name="upps", bufs=4, space="PSUM") as upps,
                tc.tile_pool(name="dnps", bufs=4, space="PSUM") as dnps,
            ):
                for e in range(E):
                    w1t = w1p.tile([P, KH, D2], BF16)
                    nc.sync.dma_start(
                        w1t[:], W1[e].rearrange("(ko p) m -> p ko m", p=P)
                    )
                    w2t = w2p.tile([P, K2, H], BF16)
                    nc.sync.dma_start(
                        w2t[:], W2[e].rearrange("(ko p) n -> p ko n", p=P)
                    )
                    xgt = xgp.tile([P, KH, cg], BF16)
                    nc.sync.dma_start(
                        xgt[:],
                        xgT[:, e * cg : (e + 1) * cg].rearrange(
                            "(ko p) n -> p ko n", p=P
                        ),
                    )
                    b1t = b1p.tile([P, M2], F32)
                    nc.sync.dma_start(b1t[:], b1[e].rearrange("(mo p) -> p mo", p=P))
                    b2t = b1p.tile([P, H], F32)
                    nc.sync.dma_start(b2t[:], _bcast_row(b2[e], P))

                    ht = hp.tile([P, K2, cg], BF16)
                    # up-projection: h[m-chunk, tokens] = W1.T @ xgT
                    for m in range(M2):
                        for noff, nsz in _chunks(cg, 512):
                            ps = upps.tile([P, 512], F32)
                            for k in range(KH):
                                nc.tensor.matmul(
                                    ps[:, :nsz],
                                    lhsT=w1t[:, k, ts(m, P)],
                                    rhs=xgt[:, k, noff : noff + nsz],
                                    start=(k == 0),
                                    stop=(k == KH - 1),
                                )
                            nc.scalar.activation(
                                ht[:, m, noff : noff + nsz],
                                ps[:, :nsz],
                                AFT.Gelu,
                                bias=b1t[:, m : m + 1],
                            )
                    # down-projection: y[tokens, H] = h.T @ W2 (h is stationary)
                    for moff, msz in _chunks(cg, P):
                        yt = ysb.tile([P, H], BF16)
                        for n in range(H // 512):
                            ps = dnps.tile([P, 512], F32)
                            for k in range(K2):
                                nc.tensor.matmul(
                                    ps[:msz, :],
                                    lhsT=ht[:, k, moff : moff + msz],
                                    rhs=w2t[:, k, ts(n, 512)],
                                    start=(k == 0),
                                    stop=(k == K2 - 1),
                                )
                            nc.vector.tensor_add(
                                yt[:msz, ts(n, 512)],
                                ps[:msz, :],
                                b2t[:msz, ts(n, 512)],
                            )
                        nc.sync.dma_start(
                            y_dram[e * cg + moff : e * cg + moff + msz, :],
                            yt[:msz, :],
                        )

            if debug_taps:
                nc.sync.dma_start(y_dbg[:], y_dram[:])
                nc.sync.dma_start(wA_dbg[:], wA_sb[:])
                nc.sync.dma_start(wB_dbg[:], wB_sb[:])

            # ---------------- phase 3: combine + residual + LN ----------------
            with tc.tile_pool(name="cmb", bufs=3) as cp:
                iaAll = persist.tile([P, nt], I32)
                ibAll = persist.tile([P, nt], I32)
                nc.sync.dma_start(iaAll[:], idxA[:])
                nc.sync.dma_start(ibAll[:], idxB[:])
                for i in range(nt):
                    ga = cp.tile([P, H], BF16)
                    nc.gpsimd.indirect_dma_start(
                        out=ga[:],
                        out_offset=None,
                        in_=y_dram[:],
                        in_offset=bass.IndirectOffsetOnAxis(
                            ap=iaAll[:, i : i + 1], axis=0
                        ),
                    )
                    gb = cp.tile([P, H], BF16)
                    nc.gpsimd.indirect_dma_start(
                        out=gb[:],
                        out_offset=None,
                        in_=y_dram[:],
                        in_offset=bass.IndirectOffsetOnAxis(
                            ap=ibAll[:, i : i + 1], axis=0
                        ),
                    )
                    xt = cp.tile([P, H], F32)
                    nc.sync.dma_start(xt[:], xblk[ts(i, P), :])
                    res = cp.tile([P, H], F32)
                    tmp = cp.tile([P, H], F32)
                    nc.vector.tensor_scalar_mul(res[:], ga[:], wA_sb[:, i : i + 1])
                    nc.vector.tensor_scalar_mul(tmp[:], gb[:], wB_sb[:, i : i + 1])
                    nc.vector.tensor_add(res[:], res[:], tmp[:])
                    nc.vector.tensor_add(res[:], res[:], xt[:])
                    # layernorm over H
                    stats = cp.tile([P, 2, 6], F32)
                    nc.vector.bn_stats(stats[:, 0, :], res[:, 0:512])
                    nc.vector.bn_stats(stats[:, 1, :], res[:, 512:1024])
                    mv = cp.tile([P, 2], F32)
                    nc.vector.bn_aggr(mv[:], stats[:])
                    rstd = cp.tile([P, 1], F32)
                    nc.scalar.activation(rstd[:], mv[:, 1:2], AFT.Sqrt, bias=eps_t[:])
                    nc.vector.reciprocal(rstd[:], rstd[:])
                    nc.vector.tensor_scalar(
                        res[:],
                        res[:],
                        mv[:, 0:1],
                        rstd[:],
                        op0=ALU.subtract,
                        op1=ALU.mult,
                    )
                    nc.vector.tensor_mul(res[:], res[:], gam_bc[:])
                    nc.vector.tensor_add(res[:], res[:], bet_bc[:])
                    nc.sync.dma_start(out[ts(i, P), :], res[:])

    nc.compile()
    return nc


def prepare_inputs(hidden_states, Wr, W1, b1, W2, b2, gamma, beta, cg=None):
    """Host-side routing bookkeeping. Returns (in_maps, cg)."""
    x = np.asarray(hidden_states, dtype=np.float32).reshape(T, H)
    Wr = np.asarray(Wr, dtype=np.float32)
    W1 = np.asarray(W1, dtype=np.float32)
    W2 = np.asarray(W2, dtype=np.float32)
    b1 = np.asarray(b1, dtype=np.float32)
    b2 = np.asarray(b2, dtype=np.float32)
    gamma = np.asarray(gamma, dtype=np.float32)
    beta = np.asarray(beta, dtype=np.float32)

    logits = x @ Wr.T  # [T, E]
    # top-2 expert indices, largest first, ties -> lower index (matches lax.top_k)
    order = np.argsort(-logits, axis=1, kind="stable")
    e1 = order[:, 0].astype(np.int32)
    e2 = order[:, 1].astype(np.int32)

    # per (core, expert) routed counts -> capacity
    maxcnt = 0
    for c in range(NCORES):
        blk = slice(c * TBLK, (c + 1) * TBLK)
        for e in range(E):
            cnt = int(np.sum((e1[blk] == e) | (e2[blk] == e)))
            maxcnt = max(maxcnt, cnt)
    if cg is None:
        cg = max(576, ((maxcnt + 31) // 32) * 32)
    assert maxcnt <= cg, (maxcnt, cg)

    W1b = np.ascontiguousarray(W1).astype(ml_dtypes.bfloat16)
    W2b = np.ascontiguousarray(W2).astype(ml_dtypes.bfloat16)
    WrT = np.ascontiguousarray(Wr.T)
    nt = TBLK // P

    in_maps = []
    for c in range(NCORES):
        t0 = c * TBLK
        xb = x[t0 : t0 + TBLK]  # [TBLK, H]
        e1b = e1[t0 : t0 + TBLK]
        e2b = e2[t0 : t0 + TBLK]
        xg = np.zeros((E * cg, H), dtype=np.float32)
        iA = np.zeros(TBLK, dtype=np.int32)
        iB = np.zeros(TBLK, dtype=np.int32)
        for e in range(E):
            sel = np.where((e1b == e) | (e2b == e))[0]
            rows = e * cg + np.arange(len(sel), dtype=np.int32)
            xg[rows] = xb[sel]
            isA = e1b[sel] == e
            iA[sel[isA]] = rows[isA]
            iB[sel[~isA]] = rows[~isA]
        in_maps.append(
            {
                "xgT": np.ascontiguousarray(xg.T).astype(ml_dtypes.bfloat16),
                "xblkT": np.ascontiguousarray(xb.T),
                "xblk": np.ascontiguousarray(xb),
                "WrT": WrT,
                "W1": W1b,
                "W2": W2b,
                "b1": np.ascontiguousarray(b1),
                "b2": np.ascontiguousarray(b2),
                "gamma": gamma,
                "beta": beta,
                "idxA": np.ascontiguousarray(iA.reshape(nt, P).T),
                "idxB": np.ascontiguousarray(iB.reshape(nt, P).T),
            }
        )
    return in_maps, cg


_COMPILED = {}


def _get_nc(cg):
    key = (TBLK, cg)
    if key not in _COMPILED:
        _COMPILED[key] = build_moe_nc(tblk=TBLK, cg=cg)
    return _COMPILED[key]


def run(inputs, trace=False):
    """Run the kernel; returns (output [B,S,H] fp32, BassKernelResults)."""
    in_maps, cg = prepare_inputs(**inputs)
    nc = _get_nc(cg)
    res = bass_utils.run_bass_kernel_spmd(
        nc, in_maps, core_ids=list(range(NCORES)), trace=trace
    )
    out = np.concatenate(
        [res.results[c]["out"] for c in range(NCORES)], axis=0
    ).reshape(B, S, H)
    return np.ascontiguousarray(out, dtype=np.float32), res


def kernel(**inputs):
    out, _ = run(inputs, trace=False)
    return out
